# revision 1
# baseline (speedup 1.0000x reference)
"""KmeansAttention Trainium2 Bass kernel — full-input contract.

Shapes (hardcoded per spec):
  qk:          (4, 16, 8192, 64) f32
  v:           (4, 16, 8192, 64) f32
  means:       (16, 64, 64)      f32
  rel_weights: (128, 16, 64)     f32
Output:        (4, 16, 8192, 64) f32

Sharding: 2 heads per core across 8 cores. All routing / k-means update /
gather / attention / scatter is per-head (the k-means reduction is over
batch only, and each core owns all 4 batches of its heads), so the cores
run fully independently — no collectives.

Per-core device pipeline, per head:
  R: stream qk, l2-normalize, build k_normT via PE transposes, sim = k_norm
     @ meansT (PE), argmax via rowmax+is_ge onehot, cluster stats via
     onehot-matmul accumulated in PSUM over all 4 batches.
  U: means update (l2norm of sums, keep old mean for empty clusters).
  D: distsT (c, t) via stationary matmuls against updated means.
  T: exact top-128 per cluster: per-512-chunk top-24 candidates via
     max8/match_replace, global top-128 over candidates -> exact threshold,
     selection mask, masked-iota index extraction (index-ascending).
  A: per 128-token window: gather qk/v rows (indirect DMA), attention with
     relative-position shift (sheared DRAM round-trip; diagonal masked by
     poisoning qrel column 127), softmax, bo matmul; the scatter-mean
     denominator (selection counts via matmul over the mask) is folded into
     the bo scale; scatter-add (DMA compute-op) into the per-(b,h) output.
"""

import sys

if "/opt/trn_rl_repo" not in sys.path:
    sys.path.insert(0, "/opt/trn_rl_repo")

import numpy as np

B, H, T, D = 4, 16, 8192, 64
WSZ, C = 128, 64
NW = T // WSZ            # 64 windows / clusters
NT = T // 128            # 64 token tiles
N_CORES = 8
HPC = H // N_CORES       # 2 heads per core
SCALE = float(D) ** -0.5
NEG = -50000.0
CHUNK = 512              # top-k chunk width
NCH = T // CHUNK         # 16 chunks
CAND = 24                # candidates kept per chunk (3 rounds of max8)
RND = CAND // 8

_CACHE = {}


def _build():
    import concourse.bass as bass
    import concourse.bacc as bacc
    import concourse.mybir as mybir
    from concourse.tile import TileContext
    from concourse.masks import make_identity

    F32 = mybir.dt.float32
    I32 = mybir.dt.int32
    AX = mybir.AxisListType
    ALU = mybir.AluOpType
    ACT = mybir.ActivationFunctionType

    nc = bacc.Bacc("TRN2", target_bir_lowering=False, debug=False,
                   num_devices=N_CORES)

    qkv_d = nc.dram_tensor("qkv", [B, HPC, T, 2 * D], F32,
                           kind="ExternalInput").ap()
    means_d = nc.dram_tensor("means", [HPC, C, D], F32, kind="ExternalInput").ap()
    rel_d = nc.dram_tensor("rel", [WSZ, HPC, D], F32, kind="ExternalInput").ap()

    # one output tensor per (b, h): offset-0 indirect-scatter targets with
    # independent dependency chains
    out_d = {}
    num2_d = {}
    for b in range(B):
        for hs in range(HPC):
            out_d[(b, hs)] = nc.dram_tensor(
                f"out_{b}_{hs}", [T, D], F32, kind="ExternalOutput").ap()
            num2_d[(b, hs)] = nc.dram_tensor(
                f"num2_{b}_{hs}", [T, D], F32).ap()

    qkv_flat = qkv_d.rearrange("b h t d -> (b h t) d")

    # internal DRAM
    shear_d = nc.dram_tensor("shear", [NW, WSZ, 2 * WSZ], F32).ap()
    denom_d = nc.dram_tensor("denom", [B, HPC, T], F32).ap()
    denom_flat = denom_d.rearrange("b h t -> (b h t)")
    sh_flat = shear_d.rearrange("w p f -> (w p f)")

    with TileContext(nc) as tc:
        with (
            tc.tile_pool(name="const", bufs=1) as cpool,
            tc.tile_pool(name="head", bufs=1) as hpool,
            tc.tile_pool(name="kno", bufs=1) as knopool,
            tc.tile_pool(name="knt", bufs=1) as kntpool,
            tc.tile_pool(name="dst", bufs=1) as dstpool,
            tc.tile_pool(name="scr", bufs=1) as scrpool,
            tc.tile_pool(name="tk", bufs=2) as tkpool,
            tc.tile_pool(name="att", bufs=3) as apool,
            tc.tile_pool(name="ps_big", bufs=5, space="PSUM") as psA,
            tc.tile_pool(name="ps_bo", bufs=2, space="PSUM") as psB,
            tc.tile_pool(name="ps_stat", bufs=1, space="PSUM") as psS,
        ):
            # ---------------- setup ----------------
            ident = cpool.tile([128, 128], F32, tag="ident")
            make_identity(nc, ident[:])

            ztile = knopool.tile([128, NT * D], F32, tag="kno")
            nc.vector.memset(ztile[:], 0.0)
            for b in range(B):
                for hs in range(HPC):
                    for od in (out_d, num2_d):
                        dst = od[(b, hs)].rearrange("(i p) d -> p i d", p=128)
                        nc.sync.dma_start(out=dst, in_=ztile[:].rearrange(
                            "p (i d) -> p i d", d=D))
            for w in range(NW):
                nc.sync.dma_start(out=shear_d[w, :, WSZ:],
                                  in_=ztile[:, :WSZ])
            nc.sync.dma_start(
                out=denom_d.rearrange("b h (x y) -> (b h x) y", y=512),
                in_=ztile[:, :512])

            # indicator for denom colsum: rows 0-63 -> col0, 64-127 -> col1
            indc = cpool.tile([128, 2], F32, tag="indc")
            nc.vector.memset(indc[:], 0.0)
            nc.vector.memset(indc[:64, 0:1], 1.0)
            nc.vector.memset(indc[64:, 1:2], 1.0)

            for hs in range(HPC):
                # ---------------- per-head constants ----------------
                meansC = hpool.tile([C, D], F32, tag="meansC")
                nc.sync.dma_start(out=meansC[:], in_=means_d[hs])
                mt_ps = psA.tile([D, C], F32, space="PSUM", tag="big")
                nc.tensor.transpose(out=mt_ps[:], in_=meansC[:],
                                    identity=ident[:64, :64])
                meansT = hpool.tile([128, C], F32, tag="meansT")
                nc.vector.tensor_copy(out=meansT[:D, :], in_=mt_ps[:])
                nc.sync.dma_start(out=meansT[D:, :], in_=meansT[:D, :])

                relw = hpool.tile([WSZ, D], F32, tag="relw")
                nc.sync.dma_start(out=relw[:], in_=rel_d[:, hs, :])
                rt_ps = psA.tile([D, WSZ], F32, space="PSUM", tag="big")
                nc.tensor.transpose(out=rt_ps[:], in_=relw[:],
                                    identity=ident[:])
                relTw = hpool.tile([D, WSZ], F32, tag="relTw")
                nc.vector.tensor_copy(out=relTw[:], in_=rt_ps[:])

                # ---------------- Phase R: routing stats ----------------
                stats = psS.tile([C, D], F32, space="PSUM", tag="stats")
                knts = [kntpool.tile([128, T], F32, tag=f"knt{p}",
                                     name=f"knt{p}_{hs}")
                        for p in range(2)]

                for b in range(B):
                    kn = knopool.tile([128, NT * D], F32, tag="kno")
                    src = qkv_d[b, hs, :, :D].rearrange(
                        "(i p) d -> p i d", p=128)
                    nc.sync.dma_start(
                        out=kn[:].rearrange("p (i d) -> p i d", d=D), in_=src)

                    kview = kn[:].rearrange("p (i d) -> p i d", d=D)
                    sq = scrpool.tile([128, NT, D], F32, tag="scr")
                    nc.vector.tensor_tensor(out=sq[:], in0=kview,
                                            in1=kview, op=ALU.mult)
                    sumsq = knopool.tile([128, NT], F32, tag="sumsq")
                    nc.vector.tensor_reduce(out=sumsq[:], in_=sq[:],
                                            axis=AX.X, op=ALU.add)
                    rsq = knopool.tile([128, NT], F32, tag="rsq")
                    nc.scalar.activation(out=rsq[:], in_=sumsq[:],
                                         func=ACT.Sqrt)
                    nc.vector.reciprocal(out=rsq[:], in_=rsq[:])
                    nc.vector.tensor_tensor(
                        out=kview, in0=kview,
                        in1=rsq[:, :, None].to_broadcast([128, NT, D]),
                        op=ALU.mult)

                    knt = knts[b // 2]
                    poff = 64 * (b % 2)
                    for i in range(0, NT, 2):
                        tp = psA.tile([128, 128], F32, space="PSUM", tag="big")
                        nc.tensor.transpose(
                            out=tp[:],
                            in_=kn[:, i * D:(i + 2) * D],
                            identity=ident[:])
                        nc.vector.tensor_copy(
                            out=knt[poff:poff + 64, i * 128:(i + 1) * 128],
                            in_=tp[:64, :])
                        nc.scalar.activation(
                            out=knt[poff:poff + 64,
                                    (i + 1) * 128:(i + 2) * 128],
                            in_=tp[64:, :], func=ACT.Copy)

                    for i in range(0, NT, 4):
                        simg = psB.tile([128, 4 * C], F32, space="PSUM",
                                        tag="bo")
                        for ti in range(4):
                            nc.tensor.matmul(
                                out=simg[:, ti * C:(ti + 1) * C],
                                lhsT=knt[poff:poff + 64,
                                         (i + ti) * 128:(i + ti + 1) * 128],
                                rhs=meansT[poff:poff + 64, :],
                                start=True, stop=True)
                        sview = simg[:].rearrange("p (t c) -> p t c", c=C)
                        mx4 = knopool.tile([128, 4], F32, tag="mx")
                        nc.vector.tensor_reduce(out=mx4[:], in_=sview,
                                                axis=AX.X, op=ALU.max)
                        ohg = knopool.tile([128, 4 * C], F32, tag="oh")
                        nc.vector.tensor_tensor(
                            out=ohg[:].rearrange("p (t c) -> p t c", c=C),
                            in0=sview,
                            in1=mx4[:, :, None].to_broadcast([128, 4, C]),
                            op=ALU.is_ge)
                        for ti in range(4):
                            first = (b == 0 and i + ti == 0)
                            last = (b == B - 1 and i + ti == NT - 1)
                            nc.tensor.matmul(
                                out=stats[:],
                                lhsT=ohg[:, ti * C:(ti + 1) * C],
                                rhs=kn[:, (i + ti) * D:(i + ti + 1) * D],
                                start=first, stop=last,
                                skip_group_check=True)

                # ---------------- Phase U: means update ----------------
                sqg = hpool.tile([C, D], F32, tag="sqg")
                ssq = hpool.tile([C, 1], F32, tag="ssq")
                nc.scalar.activation(out=sqg[:], in_=stats[:],
                                     func=ACT.Square, accum_out=ssq[:])
                rs = hpool.tile([C, 1], F32, tag="rs")
                nc.vector.tensor_scalar(rs[:], ssq[:], 1e-30, None,
                                        op0=ALU.add)
                nc.scalar.activation(out=rs[:], in_=rs[:], func=ACT.Sqrt)
                nc.vector.reciprocal(out=rs[:], in_=rs[:])
                mupd = hpool.tile([C, D], F32, tag="mupd")
                nc.vector.tensor_scalar_mul(mupd[:], stats[:], rs[:])
                bz = hpool.tile([C, 1], I32, tag="bz")
                nc.vector.tensor_scalar(bz[:], ssq[:], 1e-12, None,
                                        op0=ALU.is_lt)
                nc.vector.select(out=mupd[:], mask=bz[:].to_broadcast([C, D]),
                                 on_true=meansC[:], on_false=mupd[:])
                mu_ps = psA.tile([D, C], F32, space="PSUM", tag="big")
                nc.tensor.transpose(out=mu_ps[:], in_=mupd[:],
                                    identity=ident[:64, :64])
                meansTU = hpool.tile([128, C], F32, tag="meansTU")
                nc.vector.tensor_copy(out=meansTU[:D, :], in_=mu_ps[:])
                nc.sync.dma_start(out=meansTU[D:, :], in_=meansTU[:D, :])

                # ---------------- D/T/A per batch pair ----------------
                for pair in range(2):
                    b0 = 2 * pair
                    knt = knts[pair]
                    distsT = dstpool.tile([128, T], F32, tag="distsT")
                    for ch in range(NCH):
                        dp = psB.tile([128, CHUNK], F32, space="PSUM",
                                      tag="bo")
                        for half in range(2):
                            poff = 64 * half
                            nc.tensor.matmul(
                                out=dp[poff:poff + 64, :],
                                lhsT=meansTU[poff:poff + 64, :],
                                rhs=knt[poff:poff + 64,
                                        ch * CHUNK:(ch + 1) * CHUNK],
                                start=True, stop=True)
                        nc.vector.tensor_copy(
                            out=distsT[:, ch * CHUNK:(ch + 1) * CHUNK],
                            in_=dp[:])

                    # ---------- Phase T: exact top-128 ----------
                    scratch = scrpool.tile([128, T], F32, tag="scr")
                    nc.vector.tensor_copy(out=scratch[:], in_=distsT[:])
                    cand = tkpool.tile([128, NCH, CAND], F32, tag="cand")
                    for ch in range(NCH):
                        sl = scratch[:, ch * CHUNK:(ch + 1) * CHUNK]
                        for r in range(RND):
                            nc.vector.max(out=cand[:, ch, r * 8:(r + 1) * 8],
                                          in_=sl)
                            nc.vector.match_replace(
                                out=sl,
                                in_to_replace=cand[:, ch, r * 8:(r + 1) * 8],
                                in_values=sl, imm_value=-2.0)
                    cflat = cand[:].rearrange("p c k -> p (c k)")
                    m8 = tkpool.tile([128, 8], F32, tag="m8")
                    for r in range(WSZ // 8):
                        nc.vector.max(out=m8[:], in_=cflat)
                        nc.vector.match_replace(out=cflat, in_to_replace=m8[:],
                                                in_values=cflat,
                                                imm_value=-2.0)
                    tau = tkpool.tile([128, 1], F32, tag="tau")
                    nc.vector.tensor_copy(out=tau[:], in_=m8[:, 7:8])

                    # selection mask, in place over distsT (its last use)
                    mask = distsT
                    nc.vector.tensor_scalar(mask[:], distsT[:], tau[:], None,
                                            op0=ALU.is_ge)

                    # masked iota: (iota+1)*mask - 1, built in the scr slot
                    miota = scrpool.tile([128, T], F32, tag="scr")
                    nc.gpsimd.iota(miota[:], pattern=[[1, T]], base=1,
                                   channel_multiplier=0,
                                   allow_small_or_imprecise_dtypes=True)
                    nc.vector.tensor_tensor(out=miota[:], in0=miota[:],
                                            in1=mask[:], op=ALU.mult)
                    nc.vector.tensor_scalar(miota[:], miota[:], -1.0, None,
                                            op0=ALU.add)

                    icand = tkpool.tile([128, NCH, CAND], F32, tag="icand")
                    for ch in range(NCH):
                        sl = miota[:, ch * CHUNK:(ch + 1) * CHUNK]
                        for r in range(RND):
                            nc.vector.max(out=icand[:, ch, r * 8:(r + 1) * 8],
                                          in_=sl)
                            nc.vector.match_replace(
                                out=sl,
                                in_to_replace=icand[:, ch, r * 8:(r + 1) * 8],
                                in_values=sl, imm_value=-1.0)
                    iflat = icand[:].rearrange("p c k -> p (c k)")
                    idxf = tkpool.tile([128, WSZ], F32, tag="idxf")
                    for r in range(WSZ // 8):
                        nc.vector.max(out=m8[:], in_=iflat)
                        nc.vector.match_replace(out=iflat, in_to_replace=m8[:],
                                                in_values=iflat,
                                                imm_value=-1.0)
                        dst = idxf[:, WSZ - 8 * r - 8: WSZ - 8 * r]
                        nc.vector.tensor_copy(out=dst[:, ::-1], in_=m8[:])

                    # ---------- denom: counts -> 1/(cnt+eps), to DRAM ----
                    for ch in range(NCH):
                        dps = psB.tile([2, CHUNK], F32, space="PSUM", tag="bo")
                        nc.tensor.matmul(
                            out=dps[:], lhsT=indc[:],
                            rhs=mask[:, ch * CHUNK:(ch + 1) * CHUNK],
                            start=True, stop=True)
                        dnc = tkpool.tile([2, CHUNK], F32, tag="dnc")
                        nc.vector.tensor_scalar(dnc[:], dps[:], 1e-5, None,
                                                op0=ALU.add)
                        nc.vector.reciprocal(out=dnc[:], in_=dnc[:])
                        nc.sync.dma_start(
                            out=denom_d[b0:b0 + 2, hs,
                                        ch * CHUNK:(ch + 1) * CHUNK],
                            in_=dnc[:])

                    # ---------- window indices, transposed ----------
                    # idxf values are exact small ints in f32; shift the
                    # decremented iota back by +1? No: iota built with base=1
                    # means miota holds (t+1)*mask - 1 = t for selected.
                    it_ps = psA.tile([128, 128], F32, space="PSUM", tag="big")
                    nc.tensor.transpose(out=it_ps[:], in_=idxf[:],
                                        identity=ident[:])
                    idxT = tkpool.tile([128, 128], F32, tag="idxT")
                    nc.vector.tensor_copy(out=idxT[:], in_=it_ps[:])
                    idxTl = tkpool.tile([128, 128], I32, tag="idxTl")
                    nc.vector.tensor_copy(out=idxTl[:], in_=idxT[:])
                    for half in range(2):
                        base = float(((b0 + half) * HPC + hs) * T)
                        nc.vector.tensor_scalar(
                            idxT[:, 64 * half:64 * (half + 1)],
                            idxT[:, 64 * half:64 * (half + 1)],
                            base, None, op0=ALU.add)
                    idxTg = tkpool.tile([128, 128], I32, tag="idxTg")
                    nc.vector.tensor_copy(out=idxTg[:], in_=idxT[:])

                    # ---------- Phase A: windows ----------
                    for half in range(2):
                        b = b0 + half
                        coff = 64 * half
                        ob = out_d[(b, hs)]
                        ob2 = num2_d[(b, hs)]
                        for g in range(NW // 4):
                            w0 = 4 * g
                            qvgrp = apool.tile([128, 4, 2 * D], F32,
                                               tag="qvgrp")
                            for wi in range(4):
                                c0 = coff + w0 + wi
                                nc.gpsimd.indirect_dma_start(
                                    out=qvgrp[:, wi, :], out_offset=None,
                                    in_=qkv_flat,
                                    in_offset=bass.IndirectOffsetOnAxis(
                                        ap=idxTg[:, c0:c0 + 1], axis=0))
                            sqa = apool.tile([128, 4, D], F32, tag="sqa")
                            nc.vector.tensor_tensor(out=sqa[:],
                                                    in0=qvgrp[:, :, :D],
                                                    in1=qvgrp[:, :, :D],
                                                    op=ALU.mult)
                            rsq4 = apool.tile([128, 4], F32, tag="rsq4")
                            nc.vector.tensor_reduce(out=rsq4[:], in_=sqa[:],
                                                    axis=AX.X, op=ALU.add)
                            nc.scalar.activation(out=rsq4[:], in_=rsq4[:],
                                                 func=ACT.Sqrt)
                            nc.vector.reciprocal(out=rsq4[:], in_=rsq4[:])
                            kkg = apool.tile([128, 4, D], F32, tag="kkg")
                            nc.vector.tensor_tensor(
                                out=kkg[:], in0=qvgrp[:, :, :D],
                                in1=rsq4[:, :, None].to_broadcast(
                                    [128, 4, D]),
                                op=ALU.mult)

                            qrelg = apool.tile([128, 4, WSZ], F32,
                                               tag="qrelg")
                            dsbg = apool.tile([128, 4, WSZ], F32, tag="dsbg")
                            for wi in range(4):
                                qt_ps = psA.tile([D, 128], F32, space="PSUM",
                                                 tag="big")
                                nc.tensor.transpose(out=qt_ps[:],
                                                    in_=qvgrp[:, wi, :D],
                                                    identity=ident[:])
                                qT = apool.tile([D, 128], F32, tag="qT")
                                nc.scalar.activation(out=qT[:], in_=qt_ps[:],
                                                     func=ACT.Copy,
                                                     scale=SCALE)
                                kkt_ps = psA.tile([D, 128], F32, space="PSUM",
                                                  tag="big")
                                nc.tensor.transpose(out=kkt_ps[:],
                                                    in_=kkg[:, wi, :],
                                                    identity=ident[:])
                                kkT = apool.tile([D, 128], F32, tag="kkT")
                                nc.vector.tensor_copy(out=kkT[:],
                                                      in_=kkt_ps[:])
                                dots = psA.tile([128, 128], F32, space="PSUM",
                                                tag="big")
                                nc.tensor.matmul(out=dots[:], lhsT=qT[:],
                                                 rhs=kkT[:], start=True,
                                                 stop=True)
                                nc.vector.tensor_copy(out=dsbg[:, wi, :],
                                                      in_=dots[:])
                                qrel = psA.tile([128, 128], F32, space="PSUM",
                                                tag="big")
                                nc.tensor.matmul(out=qrel[:], lhsT=qT[:],
                                                 rhs=relTw[:], start=True,
                                                 stop=True)
                                nc.scalar.activation(out=qrelg[:, wi, :],
                                                     in_=qrel[:],
                                                     func=ACT.Copy)
                            nc.vector.memset(qrelg[:, :, 127:], NEG)
                            nc.sync.dma_start(
                                out=shear_d[w0:w0 + 4, :, :WSZ].rearrange(
                                    "w p f -> p w f"),
                                in_=qrelg[:])
                            relg = apool.tile([128, 4, WSZ], F32, tag="relg")
                            srcap = bass.AP(
                                tensor=sh_flat.tensor,
                                offset=w0 * WSZ * 2 * WSZ + 127,
                                ap=[[2 * WSZ - 1, 128],
                                    [WSZ * 2 * WSZ, 4], [1, 128]])
                            nc.sync.dma_start(out=relg[:], in_=srcap)

                            nc.vector.tensor_tensor(out=dsbg[:], in0=dsbg[:],
                                                    in1=relg[:], op=ALU.add)
                            ngm4 = apool.tile([128, 4], F32, tag="ngm4")
                            nc.vector.tensor_reduce(out=ngm4[:], in_=dsbg[:],
                                                    axis=AX.X, op=ALU.max,
                                                    negate=True)
                            smg = apool.tile([128, 4], F32, tag="smg")
                            exg = apool.tile([128, 4, 128], F32, tag="exg")
                            for wi in range(4):
                                nc.scalar.activation(out=exg[:, wi, :],
                                                     in_=dsbg[:, wi, :],
                                                     func=ACT.Exp,
                                                     bias=ngm4[:, wi:wi + 1],
                                                     accum_out=smg[:,
                                                                   wi:wi + 1])
                            rcg = apool.tile([128, 4], F32, tag="rcg")
                            nc.vector.reciprocal(out=rcg[:], in_=smg[:])
                            for wi in range(4):
                                c0 = coff + w0 + wi
                                at_ps = psA.tile([128, 128], F32, space="PSUM",
                                                 tag="big")
                                nc.tensor.transpose(out=at_ps[:],
                                                    in_=exg[:, wi, :],
                                                    identity=ident[:])
                                at_sb = apool.tile([128, 128], F32,
                                                   tag="at_sb")
                                nc.scalar.activation(out=at_sb[:],
                                                     in_=at_ps[:],
                                                     func=ACT.Copy)
                                bo_ps = psB.tile([128, D], F32, space="PSUM",
                                                 tag="bo")
                                nc.tensor.matmul(out=bo_ps[:], lhsT=at_sb[:],
                                                 rhs=qvgrp[:, wi, D:],
                                                 start=True, stop=True)
                                bo_sb = apool.tile([128, D], F32, tag="bo_sb")
                                nc.vector.tensor_scalar_mul(
                                    bo_sb[:], bo_ps[:], rcg[:, wi:wi + 1])
                                nc.gpsimd.indirect_dma_start(
                                    out=(ob if wi % 2 == 0 else ob2)[:],
                                    out_offset=bass.IndirectOffsetOnAxis(
                                        ap=idxTl[:, c0:c0 + 1], axis=0),
                                    in_=bo_sb[:], in_offset=None,
                                    compute_op=ALU.add)

                        # scatter-mean divide: out *= 1/(count+eps), one
                        # big elementwise pass per (b, h)
                        dnf = knopool.tile([128, NT], F32, tag="sumsq")
                        nc.sync.dma_start(
                            out=dnf[:],
                            in_=denom_d[b, hs].rearrange("(i p) -> p i",
                                                         p=128))
                        ov = knopool.tile([128, NT * D], F32, tag="kno")
                        ov2 = scrpool.tile([128, NT * D], F32, tag="scr")
                        obv = ob.rearrange("(i p) d -> p i d", p=128)
                        ob2v = ob2.rearrange("(i p) d -> p i d", p=128)
                        nc.sync.dma_start(
                            out=ov[:].rearrange("p (i d) -> p i d", d=D),
                            in_=obv)
                        nc.sync.dma_start(
                            out=ov2[:, :NT * D].rearrange(
                                "p (i d) -> p i d", d=D),
                            in_=ob2v)
                        nc.vector.tensor_add(out=ov[:], in0=ov[:],
                                             in1=ov2[:, :NT * D])
                        nc.vector.tensor_tensor(
                            out=ov[:].rearrange("p (i d) -> p i d", d=D),
                            in0=ov[:].rearrange("p (i d) -> p i d", d=D),
                            in1=dnf[:, :, None].to_broadcast([128, NT, D]),
                            op=ALU.mult)
                        nc.sync.dma_start(
                            out=obv,
                            in_=ov[:].rearrange("p (i d) -> p i d", d=D))

    nc.compile()
    return nc


def _get_program():
    if "nc" not in _CACHE:
        _CACHE["nc"] = _build()
    return _CACHE["nc"]


def kernel(qk, v, means, rel_weights):
    from concourse.bass_utils import run_bass_kernel_spmd

    qk = np.asarray(qk, np.float32)
    v = np.asarray(v, np.float32)
    means = np.ascontiguousarray(means, np.float32)
    rel_weights = np.ascontiguousarray(rel_weights, np.float32)
    qkv = np.concatenate([qk, v], axis=-1)  # (B, H, T, 2D)

    nc = _get_program()
    in_maps = []
    for c in range(N_CORES):
        h0 = c * HPC
        in_maps.append({
            "qkv": np.ascontiguousarray(qkv[:, h0:h0 + HPC]),
            "means": np.ascontiguousarray(means[h0:h0 + HPC]),
            "rel": np.ascontiguousarray(rel_weights[:, h0:h0 + HPC]),
        })
    res = run_bass_kernel_spmd(nc, in_maps, list(range(N_CORES)))
    out = np.empty((B, H, T, D), np.float32)
    for c in range(N_CORES):
        for b in range(B):
            for hs in range(HPC):
                out[b, c * HPC + hs] = res.results[c][f"out_{b}_{hs}"]
    return out

